# revision 31
# baseline (speedup 1.0000x reference)
"""Trainium2 Bass kernel for nn_AttnMech (sparse_attention, no-softmax attention).

Math (reference):
  q/k/v = 2x2-stride-2 convs of pose/app_pose/app  -> [B, 4*64, 48, 48]
  attn  = (Q^T K)/8 (no softmax);  out = attn @ V^T
  out   = gamma_h * out; nearest-upsample 2x; concat with pose; 1x1 conv.

Key algebraic restructure (linear attention => reassociate):
  out_h = V_h (Q_h^T K_h)^T / 8 = (V_h K_h^T) Q_h / 8 = G_h Q_h / 8
with G_h = V_h K_h^T a tiny 64x64 Gram matrix.  The per-head projection,
upsample and final 1x1 conv then fold into:
  final = fw1 @ pose_enc + up2x( W_cat @ Q + fb ) ,
  W_cat[:, 64h:64h+64] = (gamma_h/8) * fw2_h @ G_h
so the huge [2304,2304] attention matrices never exist.

Sharding over 8 cores: core c = (image b = c//2, spatial half = c%2).
Each core convs its half of the image; partial Gram matrices are
AllReduced across the core pair (64 KB); everything else is local.

This revision (v2) restructures for overlap:
  - K/V convs run in TRANSPOSED orientation (stationary = image taps,
    moving = weights) so conv output lands as [px, oc] and feeds the
    Gram matmuls directly - no PE transposes at all.  Conv bias is a
    row-replicated add folded into the PSUM->SBUF evacuation.
  - The nearest-neighbour 2x upsample is pre-applied to Q (cheap DVE
    copies) so the attention term accumulates into the SAME output
    tiles as the fw1 @ pose term via plain matmuls; the old 35us DVE
    upsample/add tail is gone.  Output tiles stream to HBM as they
    finish.
  - All input DMAs issue at t=0 on the two HWDGE queues, ordered so
    each compute phase's data arrives just in time.
  - A short burst of warmup matmuls on a memset tile defeats the PE
    HAM cold clock before the first conv.
"""

import os
import sys

for _p in ("/opt/trn_rl_repo", "/root/.axon_site/_ro/trn_rl_repo"):
    if os.path.isdir(_p) and _p not in sys.path:
        sys.path.insert(0, _p)

import numpy as np

import concourse.mybir as mybir
import concourse.tile as tile
from concourse import bacc, bass2jax

F32 = mybir.dt.float32
BF16 = mybir.dt.bfloat16
ADD = mybir.AluOpType.add
IDENT = mybir.ActivationFunctionType.Identity

P = 128          # partitions
C = 256          # channels
W_IMG = 96       # full-res width
RH = 48          # rows per half (full-res)
FR = RH * W_IMG  # 4608 flat half-image
NT = 3           # strips (16 full-res rows / 8 ds rows each)
SLEN = 2 * 16 * W_IMG  # strip len per partition (both ic chunks) 3072
NCJ = 3          # column chunks per strip (16 ds cols each)
TW = 384         # q conv tile (8 ds rows x 48 ds cols)
NLOC = 1152      # local downsampled pixels (24 x 48)
OTW = 384        # output tile width (4 full-res rows)
NOT = FR // OTW  # 12 output tiles per oc chunk

# wpack layout (per partition, bf16 words) + separate fp32 blob
QW_O, KW_O, VW_O = 0, 2048, 4096
FW1_O, FW2_O = 6144, 6656
ID_O = 7168
WLEN = 7296
# wps (fp32 words): qb[2], fb[2], pad, kb_rep[256], vb_rep[256]
QB_O, FB_O, KBR_O, VBR_O = 0, 2, 8, 264
WSLEN = 520

STAGE_DT = BF16

_CACHED_NC = None
_RUNNER = None


def _make_runner(nc, n_cores=8):
    """Like bass2jax.run_bass_via_pjrt, but inputs are pre-placed on the
    devices (parallel transfer + aligned core start) and the jitted
    executable is cached across calls."""
    import jax
    from jax.experimental.shard_map import shard_map
    from jax.sharding import Mesh, NamedSharding, PartitionSpec

    bass2jax.install_neuronx_cc_hook()

    partition_name = (
        nc.partition_id_tensor.name if nc.partition_id_tensor else None
    )
    in_names, out_names, out_avals = [], [], []
    for alloc in nc.m.functions[0].allocations:
        if not isinstance(alloc, mybir.MemoryLocationSet):
            continue
        name = alloc.memorylocations[0].name
        if alloc.kind == "ExternalInput":
            if name != partition_name:
                in_names.append(name)
        elif alloc.kind == "ExternalOutput":
            out_avals.append(
                jax.core.ShapedArray(
                    tuple(alloc.tensor_shape), mybir.dt.np(alloc.dtype)
                )
            )
            out_names.append(name)
    n_params = len(in_names)
    all_in = tuple(in_names + out_names)
    if partition_name is not None:
        all_in = all_in + (partition_name,)

    def _body(*args):
        operands = list(args)
        if partition_name is not None:
            operands.append(bass2jax.partition_id_tensor())
        return tuple(
            bass2jax._bass_exec_p.bind(
                *operands,
                out_avals=tuple(out_avals),
                in_names=all_in,
                out_names=tuple(out_names),
                lowering_input_output_aliases=(),
                sim_require_finite=True,
                sim_require_nnan=True,
                nc=nc,
            )
        )

    devices = jax.devices()[:n_cores]
    mesh = Mesh(np.asarray(devices), ("core",))
    nspec = n_params + len(out_names)
    donate = tuple(range(n_params, nspec))
    sharded = jax.jit(
        shard_map(
            _body,
            mesh=mesh,
            in_specs=(PartitionSpec("core"),) * nspec,
            out_specs=(PartitionSpec("core"),) * len(out_names),
            check_rep=False,
        ),
        donate_argnums=donate,
        keep_unused=True,
    )
    sh = NamedSharding(mesh, PartitionSpec("core"))

    def run(in_maps):
        concat_in = [
            jax.device_put(
                np.concatenate([np.asarray(m[nm]) for m in in_maps], axis=0), sh
            )
            for nm in in_names
        ]
        import jax.numpy as jnp

        concat_zeros = [
            jax.device_put(
                jnp.zeros((n_cores * a.shape[0], *a.shape[1:]), a.dtype), sh
            )
            for a in out_avals
        ]
        jax.block_until_ready(concat_in)
        jax.block_until_ready(concat_zeros)
        try:
            out_arrs = sharded(*concat_in, *concat_zeros)
            jax.block_until_ready(out_arrs)
        except Exception:
            # transient runtime desync (seen on the first launch after a
            # prior process used collectives) — one retry recovers
            concat_zeros = [
                jax.device_put(
                    jnp.zeros((n_cores * a.shape[0], *a.shape[1:]), a.dtype), sh
                )
                for a in out_avals
            ]
            jax.block_until_ready(concat_zeros)
            out_arrs = sharded(*concat_in, *concat_zeros)
            jax.block_until_ready(out_arrs)
        return [
            {
                nm: np.asarray(out_arrs[i]).reshape(n_cores, *out_avals[i].shape)[c]
                for i, nm in enumerate(out_names)
            }
            for c in range(n_cores)
        ]

    return run


def _build():
    nc = bacc.Bacc("TRN2", target_bir_lowering=False, debug=False, num_devices=8)

    xq_d = nc.dram_tensor("xq", [P, 2, FR], BF16, kind="ExternalInput").ap()
    xk_d = nc.dram_tensor("xk", [P, NT, SLEN], BF16, kind="ExternalInput").ap()
    xv_d = nc.dram_tensor("xv", [P, NT, SLEN], BF16, kind="ExternalInput").ap()
    wpack_d = nc.dram_tensor("wpack", [P, WLEN], BF16, kind="ExternalInput").ap()
    wps_d = nc.dram_tensor("wps", [P, WSLEN], F32, kind="ExternalInput").ap()

    out_d = nc.dram_tensor("out", [P, 2, FR], BF16, kind="ExternalOutput").ap()

    gpart_d = nc.dram_tensor("g_part", [P, P], BF16).ap()
    gred_d = nc.dram_tensor("g_red", [P, P], BF16).ap()

    from concourse.tile_rust import add_dep_helper

    with tile.TileContext(nc) as tc:
        with (
            tc.tile_pool(name="const", bufs=1) as cpool,
            tc.tile_pool(name="img", bufs=1) as ipool,
            tc.tile_pool(name="kvt", bufs=3) as tpool,
            tc.tile_pool(name="out", bufs=6) as opool,
            tc.tile_pool(name="ps", bufs=6, space="PSUM") as psp,
            tc.tile_pool(name="psg", bufs=2, space="PSUM") as psg,
        ):
            # ---- persistent SBUF tiles ----
            wp = cpool.tile([P, WLEN], BF16, tag="wp")
            wps_sb = cpool.tile([P, WSLEN], F32, tag="wps")
            xk_sb = cpool.tile([P, NT, SLEN], BF16, tag="xk")
            xv_sb = cpool.tile([P, NT, SLEN], BF16, tag="xv")
            xq_sb = cpool.tile([P, 2, FR], BF16, tag="xq")
            kt_all = cpool.tile([P, NT * NCJ, C], BF16, tag="ktall")
            q_sb = cpool.tile([P, 2, NLOC], BF16, tag="q")
            qx_sb = cpool.tile([P, 2, 2 * NLOC], BF16, tag="qx")  # row-doubled
            stage = cpool.tile([P, 2, FR], STAGE_DT, tag="stage")
            gstage = cpool.tile([P, P], BF16, tag="gstage")
            g_sb = cpool.tile([P, 2, P], BF16, tag="gsb")
            w_sb = cpool.tile([P, 2, C], BF16, tag="wsb")
            warm = cpool.tile([P, P], BF16, tag="warm")

            # ---- warmup tile init on the scalar engine (boots early) ----
            nc.scalar.memzero(warm[:])

            # ---- input DMAs: all issued up front, in arrival-priority
            # order per HWDGE queue ----
            # scalar queue: weight blobs, K-conv prerequisites first
            nc.scalar.dma_start(
                wp[:, KW_O : KW_O + 2048], wpack_d[:, KW_O : KW_O + 2048]
            )
            nc.scalar.dma_start(wps_sb[:], wps_d)
            nc.scalar.dma_start(
                wp[:, VW_O : VW_O + 2048], wpack_d[:, VW_O : VW_O + 2048]
            )
            nc.scalar.dma_start(
                wp[:, QW_O : QW_O + 2048], wpack_d[:, QW_O : QW_O + 2048]
            )
            nc.scalar.dma_start(
                wp[:, FW1_O:WLEN], wpack_d[:, FW1_O:WLEN]
            )
            # sync queue: image stream in consumption order.  The first
            # strip of each conv input is split at chunk granularity so
            # the first matmuls start ~2.5us earlier.
            CL = SLEN // NCJ
            for s in range(NT):
                for cc in range(NCJ):
                    nc.sync.dma_start(
                        xk_sb[:, s, cc * CL : (cc + 1) * CL],
                        xk_d[:, s, cc * CL : (cc + 1) * CL],
                    )
            nc.sync.dma_start(xv_sb[:, 0, :CL], xv_d[:, 0, :CL])
            nc.sync.dma_start(xv_sb[:, 0, CL:], xv_d[:, 0, CL:])
            nc.sync.dma_start(xv_sb[:, 1], xv_d[:, 1])
            nc.sync.dma_start(xv_sb[:, 2], xv_d[:, 2])
            H3 = FR // 3
            for s in range(NT):
                for icc in range(2):
                    nc.sync.dma_start(
                        xq_sb[:, icc, s * H3 : (s + 1) * H3],
                        xq_d[:, icc, s * H3 : (s + 1) * H3],
                    )

            nc.gpsimd.memzero(g_sb[:])

            # views of the packed weights
            qw_v = wp[:, QW_O : QW_O + 2048].rearrange(
                "p (i d o) -> p i d o", i=2, d=4
            )
            kw_v = wp[:, KW_O : KW_O + 2048].rearrange(
                "p (i d o) -> p i d o", i=2, d=4
            )
            vw_v = wp[:, VW_O : VW_O + 2048].rearrange(
                "p (i d o) -> p i d o", i=2, d=4
            )
            fw1_v = wp[:, FW1_O : FW1_O + 512].rearrange("p (i o) -> p i o", i=2)
            fw2_v = wp[:, FW2_O : FW2_O + 512].rearrange("p (i o) -> p i o", i=2)
            id_v = wp[:, ID_O : ID_O + P]

            def sca(off):  # [P, 1] fp32 per-partition scalar view
                return wps_sb[:, off : off + 2]

            # ---- PE warmup: ~3us of matmuls on a zeroed tile so the HAM
            # clock gate opens before the first real conv ----
            for i in range(16):
                pw = psg.tile([P, P], F32, tag="pw", name=f"warm{i}")
                nc.tensor.matmul(
                    pw[:], warm[:], warm[:], start=True, stop=True
                )

            # ---- K/V convs, transposed orientation ----
            # xk/xv are packed host-side as im2col chunks: strip s holds 3
            # chunks of (icc 2, dd 4, 128 ds-pixels).  stationary = image
            # taps [ic, 128 px], moving = conv weights [ic, 256]; psum
            # result is [px, oc] - feeds the Gram directly, no transposes.
            def convT_chunk(src_sb, w_v, s, cc, nm):
                ps = psp.tile([P, C], F32, tag="ps", name=f"c{nm}{s}{cc}")
                v = src_sb[:, s].rearrange(
                    "p (c i d x) -> p c i d x", c=NCJ, i=2, d=4
                )
                first = True
                for icc in range(2):
                    for dd in range(4):
                        nc.tensor.matmul(
                            ps[:],
                            v[:, cc, icc, dd, :],
                            w_v[:, icc, dd, :],
                            start=first,
                            stop=(icc == 1 and dd == 3),
                        )
                        first = False
                return ps

            # K conv: all 9 chunks into kt_all
            for s in range(NT):
                for cc in range(NCJ):
                    t = NCJ * s + cc
                    ps = convT_chunk(xk_sb, kw_v, s, cc, "k")
                    nc.vector.tensor_tensor(
                        kt_all[:, t, :], ps[:],
                        wps_sb[:, KBR_O : KBR_O + C], ADD,
                    )

            # V conv + streamed Gram accumulation (block-diagonal halves)
            gps = [
                psg.tile([P, P], F32, tag="pw", name=f"gps{g}") for g in range(2)
            ]
            gmm = None
            for s in range(NT):
                for cc in range(NCJ):
                    t = NCJ * s + cc
                    ps = convT_chunk(xv_sb, vw_v, s, cc, "v")
                    vtt = tpool.tile([P, C], BF16, tag="vtt")
                    nc.vector.tensor_tensor(
                        vtt[:], ps[:], wps_sb[:, VBR_O : VBR_O + C], ADD
                    )
                    for g in range(2):
                        gmm = nc.tensor.matmul(
                            gps[g][:],
                            vtt[:, g * P : (g + 1) * P],
                            kt_all[:, t, g * P : (g + 1) * P],
                            start=(t == 0),
                            stop=(t == NT * NCJ - 1),
                            skip_group_check=True,
                        )

            # ---- Gram exchange: stage the 4 per-head diagonal 64-blocks
            # into [128, 128] (g in columns), AllReduce across the pair.
            # Staging + store run on the (idle) scalar engine so the
            # collective trigger fires as early as possible. ----
            for g in range(2):
                for hh in range(2):
                    r0 = 64 * hh
                    nc.vector.tensor_copy(
                        gstage[r0 : r0 + 64, 64 * g : 64 * g + 64],
                        gps[g][r0 : r0 + 64, r0 : r0 + 64],
                    )
            nc.scalar.dma_start(gpart_d, gstage[:])
            nc.gpsimd.collective_compute(
                "AllReduce",
                ADD,
                replica_groups=[[0, 1], [2, 3], [4, 5], [6, 7]],
                ins=[gpart_d],
                outs=[gred_d],
            )
            # reduced Gram lands directly in the block-diagonal slots of
            # g_sb (zeroed off-diagonal); no staging buffer needed
            for g in range(2):
                for hh in range(2):
                    r0 = 64 * hh
                    nc.scalar.dma_start(
                        g_sb[r0 : r0 + 64, g, r0 : r0 + 64],
                        gred_d[r0 : r0 + 64, 64 * g : 64 * g + 64],
                    )

            # ---- Q conv (normal orientation), fills collective latency ----
            xqv = [
                xq_sb[:, icc, :].rearrange("p (r w) -> p r w", w=W_IMG)
                for icc in range(2)
            ]
            for qcc in range(2):
                for nt in range(NT):
                    psq = psp.tile([P, TW], F32, tag="ps", name=f"q{qcc}{nt}")
                    first = True
                    for icc in range(2):
                        for dd in range(4):
                            di, dj = dd // 2, dd % 2
                            mm = nc.tensor.matmul(
                                psq[:],
                                qw_v[:, icc, dd, qcc * P : (qcc + 1) * P],
                                xqv[icc][:, 16 * nt + di : 16 * nt + 16 : 2, dj::2],
                                start=first,
                                stop=(icc == 1 and dd == 3),
                            )
                            if first and gmm is not None:
                                add_dep_helper(
                                    mm.ins, gmm.ins, sync=False,
                                    reason="pin Q conv after Gram",
                                )
                            first = False
                    nc.scalar.activation(
                        q_sb[:, qcc, nt * TW : (nt + 1) * TW], psq[:], IDENT,
                        bias=sca(QB_O)[:, qcc : qcc + 1], scale=1.0,
                    )

            # ---- expand Q 2x along rows only (contiguous copies, fast);
            # the column duplication happens inside the z matmuls via a
            # stride-0 moving AP ----
            for g in range(2):
                qv = q_sb[:, g, :].rearrange("p (r w) -> p r w", w=48)
                qxv = qx_sb[:, g, :].rearrange("p (r w) -> p r w", w=48)
                for rr in range(2):
                    nc.vector.tensor_copy(qxv[:, rr::2, :], qv[:])

            # ---- pose term into bf16 staging (fb bias folded) ----
            for occ in range(2):
                for t in range(NOT):
                    ps = psp.tile([P, OTW], F32, tag="ps", name=f"p{occ}{t}")
                    for icc in range(2):
                        nc.tensor.matmul(
                            ps[:],
                            fw1_v[:, icc, occ * P : (occ + 1) * P],
                            xq_sb[:, icc, t * OTW : (t + 1) * OTW],
                            start=(icc == 0),
                            stop=(icc == 1),
                        )
                    dst = stage[:, occ, t * OTW : (t + 1) * OTW]
                    if t % 2:
                        nc.scalar.activation(
                            dst, ps[:], IDENT,
                            bias=sca(FB_O)[:, occ : occ + 1], scale=1.0,
                        )
                    else:
                        nc.vector.tensor_tensor(
                            dst, ps[:],
                            sca(FB_O)[:, occ : occ + 1].to_broadcast([P, OTW]),
                            ADD,
                        )

            # ---- W_cat^T = blockdiag(G) @ fw2'^T (gamma/8 pre-folded) ----
            for g in range(2):
                psw = psp.tile([P, C], F32, tag="ps", name=f"w{g}")
                nc.tensor.matmul(
                    psw[:], g_sb[:, g, :], fw2_v[:, g, :], start=True, stop=True
                )
                nc.vector.tensor_copy(w_sb[:, g, :], psw[:])

            # ---- attention term + staged pose term -> output stream.
            # Each output tile is one PSUM accumulation group: an identity
            # matmul re-reads the staged pose term into PSUM, then the two
            # z matmuls accumulate the attention term on top.  The z moving
            # operand reads 4 row-doubled rows of qx with each column read
            # twice via a stride-0 broadcast dim (nearest 2x upsample for
            # free inside the matmul).  Evacuations alternate DVE/ACT and
            # the output DMA ships 4 tiles at a time. ----
            qxr = [
                qx_sb[:, g, :].rearrange("p (r w) -> p r w", w=48)
                for g in range(2)
            ]
            for occ in range(2):
                for b4 in range(NOT // 4):
                    obuf = opool.tile([P, 4 * OTW], BF16, tag="obuf")
                    for j in range(4):
                        t = 4 * b4 + j
                        ps = psp.tile([P, OTW], F32, tag="ps", name=f"z{occ}{t}")
                        stg = stage[:, occ, t * OTW : (t + 1) * OTW]
                        # 1/3 of tiles: pose term re-read via identity
                        # matmul + ACT evac; the rest: DVE adds the pose
                        # term during evacuation.  Balances PE/ACT/DVE.
                        pe_path = t % 3 == 0
                        if pe_path:
                            nc.tensor.matmul(
                                ps[:], id_v, stg, start=True, stop=False
                            )
                        for g in range(2):
                            mov = qxr[g][
                                :, 4 * t : 4 * t + 4, :, None
                            ].to_broadcast([P, 4, 48, 2])
                            nc.tensor.matmul(
                                ps[:],
                                w_sb[:, g, occ * P : (occ + 1) * P],
                                mov,
                                start=(not pe_path and g == 0),
                                stop=(g == 1),
                            )
                        dst = obuf[:, j * OTW : (j + 1) * OTW]
                        if pe_path:
                            nc.scalar.copy(dst, ps[:])
                        else:
                            nc.vector.tensor_tensor(dst, ps[:], stg, ADD)
                    nc.sync.dma_start(
                        out_d[:, occ, 4 * b4 * OTW : 4 * (b4 + 1) * OTW],
                        obuf[:],
                    )

    nc.compile()
    return nc


def _prep_inputs(inputs):
    """Build the 8 per-core input maps (host-side shard + weight packing)."""
    import ml_dtypes

    f = np.float32
    b16 = ml_dtypes.bfloat16
    qw, qb = np.asarray(inputs["qw"], f), np.asarray(inputs["qb"], f)
    kw, kb = np.asarray(inputs["kw"], f), np.asarray(inputs["kb"], f)
    vw, vb = np.asarray(inputs["vw"], f), np.asarray(inputs["vb"], f)
    gamma = np.asarray(inputs["gamma"], f)
    fw, fb = np.asarray(inputs["fw"], f), np.asarray(inputs["fb"], f)
    pose = np.asarray(inputs["pose_enc"], f)
    app_pose = np.asarray(inputs["app_pose_enc"], f)
    app = np.asarray(inputs["app_enc"], f)

    wpack = np.zeros((P, WLEN), dtype=b16)
    wps = np.zeros((P, WSLEN), dtype=f)

    def packw(dst_off, w):
        # w [oc, ic, 2, 2] -> [p, icc, dd, oc]
        t = w.transpose(1, 2, 3, 0).reshape(2, P, 4, C).transpose(1, 0, 2, 3)
        wpack[:, dst_off : dst_off + 2048] = t.reshape(P, 2048).astype(b16)

    packw(QW_O, qw)
    packw(KW_O, kw)
    packw(VW_O, vw)
    wpack[:, FW1_O : FW1_O + 512] = (
        fw[:, :C, 0, 0].T.reshape(2, P, C).transpose(1, 0, 2).reshape(P, 512)
    ).astype(b16)
    gsc = (np.repeat(gamma.astype(np.float64), 64) / 8.0)[:, None]
    fw2s = (fw[:, C:, 0, 0].T.astype(np.float64) * gsc).astype(f)
    wpack[:, FW2_O : FW2_O + 512] = (
        fw2s.reshape(2, P, C).transpose(1, 0, 2).reshape(P, 512)
    ).astype(b16)
    wpack[:, ID_O : ID_O + P] = np.eye(P, dtype=f).astype(b16)
    wps[:, QB_O : QB_O + 2] = qb.reshape(2, P).T
    wps[:, FB_O : FB_O + 2] = fb.reshape(2, P).T
    wps[:, KBR_O : KBR_O + C] = kb[None, :]
    wps[:, VBR_O : VBR_O + C] = vb[None, :]

    def shard_q(x, b, h):  # [p, icc, fr]
        halfimg = x[b, :, RH * h : RH * (h + 1), :].reshape(2, P, FR)
        return halfimg.transpose(1, 0, 2).astype(b16)

    def shard_kv(x, b, h):  # [p, strip, 3*(icc*dd*128)] im2col chunk-major
        xs = x[b, :, RH * h : RH * (h + 1), :]  # [256, 48, 96]
        taps = np.stack(
            [xs[:, di::2, dj::2] for di in (0, 1) for dj in (0, 1)], 1
        )  # [256, 4, 24, 48]
        im = taps.reshape(2, P, 4, NT * NCJ, P)  # [icc, p, dd, chunk, px]
        im = im.transpose(1, 3, 0, 2, 4)  # [p, chunk, icc, dd, px]
        return im.reshape(P, NT, SLEN).astype(b16)

    in_maps = []
    for c in range(8):
        b, h = c // 2, c % 2
        in_maps.append({
            "xq": shard_q(pose, b, h),
            "xk": shard_kv(app_pose, b, h),
            "xv": shard_kv(app, b, h),
            "wpack": wpack,
            "wps": wps,
        })
    return in_maps


def _get_runner():
    global _CACHED_NC, _RUNNER
    if _CACHED_NC is None:
        _CACHED_NC = _build()
    if _RUNNER is None:
        _RUNNER = _make_runner(_CACHED_NC)
    return _RUNNER


def _assemble(results):
    out = np.empty((4, C, W_IMG, W_IMG), dtype=np.float32)
    for c in range(8):
        b, h = c // 2, c % 2
        o = np.asarray(results[c]["out"], dtype=np.float32)  # [P, 2, FR]
        out[b, :, RH * h : RH * (h + 1), :] = o.transpose(1, 0, 2).reshape(
            C, RH, W_IMG
        )
    return out


def kernel(**inputs):
    run = _get_runner()
    in_maps = _prep_inputs(inputs)
    return _assemble(run(in_maps))
